# revision 6
# baseline (speedup 1.0000x reference)
"""Trainium2 Bass kernel for nn_Attention_75342316306884.

Per-batch channel-channel attention:
  xf = x.reshape(B, C, HW); cf = condition.reshape(B, C, HW)
  w1 = softmax(xf @ xf^T * HW^-0.5); w2 = softmax(sig(cf) @ sig(cf)^T * HW^-0.5)
  out = xf + (w1 + w2) @ xf          -> [B, C, HW] float32

Sharding: pure data parallel, batch dim 64 -> 8 cores x 8 batches.

Structure (per core):
  Phase A (per batch): cast-DMA x -> bf16 natural tiles (kept in SBUF for the
    apply matmul + residual); write them to a padded bf16 DRAM scratch;
    cast-DMA condition -> bf16, ACT sigmoid, write to scratch.  Only the
    sigmoid table set is used here.
  Phase B (per batch): hardware DMA-transpose the scratch back as [n, c]
    bf16 tiles; two 512x512 grams on TensorE (bf16, PSUM f32); ACT exp with
    fused per-row accumulation (Z); apply matmuls; per-partition
    normalization + residual on VectorE.  Only the exp table set is used.

Key algebraic trick: G = xf@xf^T is symmetric, so the *unnormalized*
E = exp(G*s) is symmetric too; softmax(G) @ xf == diag(1/rowsum(E)) @ (E @ xf).
E's stored [c-part, d-free] tiles therefore serve directly as the [K=d, M=c]
stationary operands of the apply matmul - no transpose of the attention
matrix, and the row-normalization is a per-partition scalar on E @ xf.
(exp without max-subtraction is safe: logits are bounded by ~|x|^2/28 ~ 35.)
"""

import sys

import numpy as np

for _p in ("/opt/trn_rl_repo",):
    if _p not in sys.path:
        sys.path.append(_p)

import concourse.bass as bass
import concourse.mybir as mybir
import concourse.tile as tile
from concourse.bass_utils import run_bass_kernel_spmd
from concourse.vector_clock import ScopedClock

F32 = mybir.dt.float32
BF16 = mybir.dt.bfloat16
AF = mybir.ActivationFunctionType

N_CORES = 8
B_PER_CORE = 8
C = 512  # channels
HW = 784  # 28*28
HWP = 896  # padded to 7*128 for the xbar transpose
SCALE = float(HW) ** -0.5
P = 128
N_KCH = 7  # contraction chunks: 6x128 + 1x16
KCH_SIZES = (128, 128, 128, 128, 128, 128, 16)
N_CB = 4  # 512 / 128 c-blocks
APPLY_NSPLIT = ((0, 512), (512, 272))


def _patch_tile_drain():
    """walrus codegen in this toolchain rejects >1 sem-wait on one SP CTRL
    (drain/nop) instruction; spread the Tile end-of-context drain waits
    across several nops instead."""
    if getattr(tile.TileContext, "_drain_patched", False):
        return

    def _drain_and_barrier(self, tick_clock, wait_clock):
        absorber = self.nc.sync.nop()
        wait_clock.add_sem_waits(
            absorber.ins, ScopedClock({None: tick_clock.global_clock})
        )
        si = absorber.ins.sync_info
        waits = list(si.on_wait) if si is not None and si.on_wait else []
        if len(waits) > 1:
            absorber.ins.sync_info = mybir.SyncInfo(on_wait=waits[:1], on_update=[])
            for w in waits[1:]:
                n2 = self.nc.sync.nop()
                n2.ins.sync_info = mybir.SyncInfo(on_wait=[w], on_update=[])
        self.nc.sync.drain()
        self.nc.all_engine_barrier()
        assert self.sems is not None
        popped = self.nc._tile_sem_poison_stack.pop()
        assert popped is self._sem_poison
        self.nc.clear_and_free_semaphores(list(self.sems.allocated().values()))
        self.nc.all_engine_barrier()

    tile.TileContext._drain_and_barrier = _drain_and_barrier
    tile.TileContext._drain_patched = True


def _split_multi_waits(nc, limit=1):
    """This walrus build allows only `limit` sem-wait commands per
    instruction.  Hoist excess waits onto same-engine NoOps placed
    immediately before the instruction (per-engine program order makes
    this semantically identical)."""
    n_split = 0
    for f in nc.m.functions:
        for bb in f.blocks:
            new_insts = []
            for inst in bb.instructions:
                si = inst.sync_info
                waits = list(si.on_wait) if si is not None and si.on_wait else []
                if len(waits) > limit:
                    for j, w in enumerate(waits[:-limit]):
                        nop = mybir.InstNoOp(
                            name=f"{inst.name}-wsplit{j}", ins=[], outs=[]
                        )
                        nop.engine = inst.engine
                        nop.sync_info = mybir.SyncInfo(on_wait=[w], on_update=[])
                        new_insts.append(nop)
                    inst.sync_info = mybir.SyncInfo(
                        on_wait=waits[-limit:],
                        on_update=list(si.on_update) if si.on_update else [],
                    )
                    n_split += 1
                new_insts.append(inst)
            if len(new_insts) != len(bb.instructions):
                bb.instructions = new_insts
                assert len(bb.instructions) == len(new_insts)
    return n_split


def _gram_exp(nc, psum_g, opT, e_pool, z_pool, etag):
    """opT: 7 [128, 512] bf16 tiles, [n-part, c-free]; chunk 6 only has its
    first 16 partitions valid.  Returns E = exp(scale*gram) (4 x [128, 512]
    bf16) and r = 1/rowsum(E) (4 x [128, 1] f32)."""
    es, rs = [], []
    for cb in range(N_CB):
        g = psum_g.tile([P, C], F32, tag="g")
        for k in range(N_KCH):
            kk = KCH_SIZES[k]
            nc.tensor.matmul(
                g[:],
                opT[k][:kk, cb * P : (cb + 1) * P],
                opT[k][:kk, :],
                start=(k == 0),
                stop=(k == N_KCH - 1),
            )
        e = e_pool.tile([P, C], BF16, tag=etag)
        z = z_pool.tile([P, 1], F32, tag="z")
        nc.scalar.activation(e[:], g[:], AF.Exp, scale=SCALE, accum_out=z[:])
        r = z_pool.tile([P, 1], F32, tag="r" + etag)
        nc.vector.reciprocal(r[:], z[:])
        es.append(e)
        rs.append(r)
    return es, rs


def build_kernel():
    _patch_tile_drain()
    nc = bass.Bass()
    x_ext = nc.declare_dram_parameter("x", [B_PER_CORE, C, HW], F32, isOutput=False)
    c_ext = nc.declare_dram_parameter(
        "condition", [B_PER_CORE, C, HW], F32, isOutput=False
    )
    out_ext = nc.declare_dram_parameter("out", [B_PER_CORE, C, HW], F32, isOutput=True)

    xscr = nc.dram_tensor("xscr", [B_PER_CORE, C, HWP], BF16)
    cscr = nc.dram_tensor("cscr", [B_PER_CORE, C, HWP], BF16)

    with tile.TileContext(nc) as tc:
        with (
            tc.tile_pool(name="xn", bufs=4 * B_PER_CORE) as xn_pool,
            tc.tile_pool(name="cn", bufs=3) as cn_pool,
            tc.tile_pool(name="cs", bufs=3) as cs_pool,
            tc.tile_pool(name="xT", bufs=9) as xT_pool,
            tc.tile_pool(name="csT", bufs=9) as csT_pool,
            tc.tile_pool(name="E", bufs=5) as e_pool,
            tc.tile_pool(name="z", bufs=10) as z_pool,
            tc.tile_pool(name="outs", bufs=3) as out_pool,
            tc.tile_pool(name="psum_g", bufs=2, space="PSUM") as psum_g,
            tc.tile_pool(name="psum_u", bufs=4, space="PSUM") as psum_u,
        ):
            # ---- Phase A: casts + sigmoid + scratch staging ----
            xn = {}  # (b, cb) -> bf16 natural tile, kept for phase B
            for b in range(B_PER_CORE):
                for cb in range(N_CB):
                    t = xn_pool.tile([P, HW], BF16, tag="xn")
                    nc.gpsimd.dma_start(t[:], x_ext[b, cb * P : (cb + 1) * P, :])
                    nc.sync.dma_start(xscr[b, cb * P : (cb + 1) * P, :HW], t[:])
                    xn[b, cb] = t
                for cb in range(N_CB):
                    cn = cn_pool.tile([P, HW], BF16, tag="cn")
                    nc.gpsimd.dma_start(cn[:], c_ext[b, cb * P : (cb + 1) * P, :])
                    cs = cs_pool.tile([P, HW], BF16, tag="cs")
                    nc.scalar.activation(cs[:], cn[:], AF.Sigmoid)
                    nc.sync.dma_start(cscr[b, cb * P : (cb + 1) * P, :HW], cs[:])

            # ---- Phase B: transpose-back, grams, softmax-apply ----
            for b in range(B_PER_CORE):
                xT, csT = [], []
                for k in range(N_KCH):
                    t = xT_pool.tile([P, C], BF16, tag="xT")
                    nc.sync.dma_start_transpose(t[:], xscr[b, :, k * P : (k + 1) * P])
                    xT.append(t)
                    s = csT_pool.tile([P, C], BF16, tag="csT")
                    nc.sync.dma_start_transpose(s[:], cscr[b, :, k * P : (k + 1) * P])
                    csT.append(s)

                E1, r1 = _gram_exp(nc, psum_g, xT, e_pool, z_pool, "e1")
                E2, r2 = _gram_exp(nc, psum_g, csT, e_pool, z_pool, "e2")

                for cb in range(N_CB):
                    for n0, nw in APPLY_NSPLIT:
                        u1 = psum_u.tile([P, nw], F32, tag="u")
                        for k in range(N_CB):
                            nc.tensor.matmul(
                                u1[:],
                                E1[k][:, cb * P : (cb + 1) * P],
                                xn[b, k][:, n0 : n0 + nw],
                                start=(k == 0),
                                stop=(k == N_CB - 1),
                            )
                        u2 = psum_u.tile([P, nw], F32, tag="u")
                        for k in range(N_CB):
                            nc.tensor.matmul(
                                u2[:],
                                E2[k][:, cb * P : (cb + 1) * P],
                                xn[b, k][:, n0 : n0 + nw],
                                start=(k == 0),
                                stop=(k == N_CB - 1),
                            )
                        t1 = out_pool.tile([P, nw], F32, tag="t1")
                        nc.vector.tensor_scalar(
                            t1[:], u1[:], r1[cb][:], None, mybir.AluOpType.mult
                        )
                        t2 = out_pool.tile([P, nw], F32, tag="t2")
                        nc.vector.tensor_scalar(
                            t2[:], u2[:], r2[cb][:], None, mybir.AluOpType.mult
                        )
                        t3 = out_pool.tile([P, nw], F32, tag="t3")
                        nc.vector.tensor_add(t3[:], t1[:], t2[:])
                        o = out_pool.tile([P, nw], F32, tag="o")
                        nc.vector.tensor_add(o[:], t3[:], xn[b, cb][:, n0 : n0 + nw])
                        nc.sync.dma_start(
                            out_ext[b, cb * P : (cb + 1) * P, n0 : n0 + nw], o[:]
                        )
    n = _split_multi_waits(nc)
    print(f"[kernel] split {n} multi-wait instructions")
    return nc


_NC_CACHE = None


def kernel(x: np.ndarray, condition: np.ndarray, _trace: bool = False):
    """Full inputs [64, 512, 28, 28] f32 -> full output [64, 512, 784] f32."""
    global _NC_CACHE
    B = x.shape[0]
    xf = np.ascontiguousarray(x.reshape(B, C, HW), dtype=np.float32)
    cf = np.ascontiguousarray(condition.reshape(B, C, HW), dtype=np.float32)

    if _NC_CACHE is None:
        _NC_CACHE = build_kernel()
    nc = _NC_CACHE

    in_maps = [
        {
            "x": xf[i * B_PER_CORE : (i + 1) * B_PER_CORE],
            "condition": cf[i * B_PER_CORE : (i + 1) * B_PER_CORE],
        }
        for i in range(N_CORES)
    ]
    res = run_bass_kernel_spmd(nc, in_maps, core_ids=list(range(N_CORES)), trace=_trace)
    out = np.concatenate([res.results[i]["out"] for i in range(N_CORES)], axis=0)
    kernel.last_result = res
    return out
